# revision 22
# baseline (speedup 1.0000x reference)
"""Position-attention kernel for Trainium2 (8 NeuronCores, SPMD).

Reference computation (per batch b):
    q = Wq @ x + bq        [32, 4096]
    k = Wk @ x + bk        [32, 4096]
    v = Wv @ x + bv        [256, 4096]
    attn = softmax_j(q_i . k_j)           [4096, 4096]
    out[c, i] = sum_j v[c, j] attn[i, j]
    y = gamma * out + x

Sharding: B=4 batches x 2 query-halves -> 8 cores. Each core computes the
full softmax rows for its 2048 queries against all 4096 keys of its batch.
Host rotates x columns per core so the core's query half is always columns
0:2048 (softmax and the PV contraction are invariant to key/value column
order, as long as K and V use the same order).

Device-side structure (per core):
  - projections in bf16 (x pre-cast on host; weights pre-packed on host).
  - K projection col-tiled: stationary WkT [128, 32] at tile_position
    (0, 32*(j%4)) so four 32-row key-block outputs land packed in one PSUM
    tile; kf stored [32*(j%4)+d, j//4, :].
  - scores computed transposed (sT[j, i]) in PSUM, 2 key-blocks at a time
    packed into PE row-groups 32*(j%4) via tile_position (the K=32
    contractions run concurrently); q replicated into all four row groups.
  - exp is split across two engines: ACT does half the groups (exact exp),
    DVE the other half with a Schraudolph fast-exp (one tensor_scalar:
    int16 bits = round(s*A + B), bitcast to bf16; +-3% sawtooth that
    largely cancels in the softmax ratio). Removes ACT as co-bottleneck.
  - PV: out[i, c] = sum_j e[j, i] * vT[j, c] with e-blocks stationary;
    vT carries an all-ones column so column 256 of the output is the
    softmax denominator (per-partition = per-query).
  - head is chunk-driven (one 512-col x DMA per k-proj quad) and the PV of
    st0's first two row-blocks CHASES the exp pipeline so the PE stream
    never breaks: the HAM activity monitor grants full clock only under
    sustained dense matmul activity, and a failed window costs a long
    half-clock lockout.
  - epilogue: y_T[i, :] = out[i, :] * (gamma / sum_i) + (x_T[i, :] +
    gamma * bv); bv-folding works because sum_j attn = 1. Output written
    transposed; host transposes back (pure layout).
"""

import os
import numpy as np

P = 128
B = 4
C = 256
CQ = 32
H = W = 64
N = H * W            # 4096 keys per batch
NH = N // 2          # 2048 queries per core
NCB = C // P         # 2 channel blocks
ST = 512             # query supertile
NST = NH // ST       # 4
JB = N // P          # 32 key blocks
NQG = JB // 4        # 8 k-proj quad groups
NGR0 = JB // 2       # 16 score pair-groups for st0 (2-bank PSUM tiles)
# st1-3 use triples (3-bank tiles, fewer per-burst overheads)
GROUPS3 = [list(range(3 * g, min(3 * g + 3, 32))) for g in range(11)]
NGR = len(GROUPS3)

# exp engine assignment: which groups go to the DVE (Schraudolph).
# st0's DVE is also busy with v-casts and bias adds, so it gets fewer,
# spread out so the ib0/ib1 chase is never starved.
DVE_G0 = {2, 5, 8, 10, 12, 14}
DVE_G = {7, 8, 9, 10}

# Schraudolph fast-exp constants: bf16 bits = round(s * SCHRA_A + SCHRA_B)
SCHRA_A = 184.66496523378732          # 128 * log2(e)
SCHRA_B = 16250.515                    # centered: 127*128 - 128*0.0602/(2 ln2)

_PROG = None         # cached build
LAST_RESULT = None   # BassKernelResults of the last run (for test harness)


def _build_program():
    import concourse.mybir as mybir
    import concourse.tile as tile
    from concourse import bacc
    from concourse.bass import ds

    fp32 = mybir.dt.float32
    bf16 = mybir.dt.bfloat16
    i16 = mybir.dt.int16

    nc = bacc.Bacc(None, target_bir_lowering=False, debug=False)

    xb_d = nc.declare_dram_parameter("xb", [C, N], bf16, isOutput=False)
    # xpb = xT + gamma*bv, already in SBUF layout [p, row_block*C]
    xpb_d = nc.declare_dram_parameter("xpb", [P, (NH // P) * C], fp32, isOutput=False)
    wq_d = nc.declare_dram_parameter("wq_pre", [P, NCB * P], bf16, isOutput=False)
    wk_d = nc.declare_dram_parameter("wkT_pre", [P, NCB * CQ], bf16, isOutput=False)
    wv_d = nc.declare_dram_parameter("wv_pre", [P, NCB * C], bf16, isOutput=False)
    # consts columns: 0=bq_rep, 1=bk_pack, 2=gamma, 3=pad (one DMA trigger)
    cst_d = nc.declare_dram_parameter("consts", [P, 4], fp32, isOutput=False)
    yT_d = nc.declare_dram_parameter("yT", [NH, C], fp32, isOutput=True)

    from contextlib import ExitStack

    with tile.TileContext(nc) as tc:
        with (
            tc.tile_pool(name="singles", bufs=1) as singles,
            tc.tile_pool(name="epool", bufs=30) as epool,
            tc.tile_pool(name="stpool", bufs=4) as stpool,
            tc.tile_pool(name="ivpool", bufs=4) as ivpool,
        ):
            # PSUM, head phase (8 banks): score pair tiles 2x2, proj pool 2,
            # and two dedicated banks for the st0 ib0/ib1 PV accumulators.
            # After the head these are all released and replaced by the
            # steady-state layout: score triple tiles 2x3 + rotating out 2.
            head_ps = ExitStack()
            pp_mm_h = head_ps.enter_context(
                tc.tile_pool(name="pp_mm_h", bufs=2, space="PSUM"))
            pp_pj = head_ps.enter_context(
                tc.tile_pool(name="pp_pj", bufs=2, space="PSUM"))
            out_a, _free_a = tc.tile([P, C + 1], fp32, space="PSUM", name="out_a")
            out_b, _free_b = tc.tile([P, C + 1], fp32, space="PSUM", name="out_b")
            outs = [out_a, out_b]

            # ---- persistent SBUF tensors ----
            xb_sb = singles.tile([P, NCB, N], bf16)
            xpb_sb = singles.tile([P, NH // P, C], fp32)  # xT + gamma*bv
            wq_sb = singles.tile([P, NCB, P], bf16)
            wkT_sb = singles.tile([P, NCB, CQ], bf16)
            wv_sb = singles.tile([P, NCB, C], bf16)
            cst_sb = singles.tile([P, 4], fp32)
            bq_sb = cst_sb[:, 0:1]
            bk_sb = cst_sb[:, 1:2]
            gm_sb = cst_sb[:, 2:3]
            kf_sb = singles.tile([P, NQG, P], bf16)  # row 32*(j%4)+d, quad j//4
            q_sb = singles.tile([P, NH], bf16)       # q replicated in 4 groups
            vT_sb = singles.tile([P, JB, C + 1], bf16)  # col C is all-ones

            # dummy exp pre-loads the ACT function table (~2.7us) so the
            # first real exp doesn't pay it. No PE warmup: tripping the HAM
            # before activity can be SUSTAINED causes a half-clock lockout.
            warm_e = singles.tile([1, 1], fp32)
            nc.vector.memset(warm_e[:], 0.0)
            nc.scalar.activation(
                warm_e, warm_e, mybir.ActivationFunctionType.Exp
            )

            # ---- input DMAs. Each dma_start trigger occupies its queue for
            # ~600ns and transfers on one HWDGE ring serialize, so use FEW,
            # LARGE transfers (>=2KB per partition line for full SDMA
            # efficiency) spread over BOTH HWDGE rings (sync + scalar): xb
            # in four 1024-col chunks, alternating rings. xpb (residual)
            # last: first use is the st0/ib0 epilogue.
            nc.sync.dma_start(
                out=wkT_sb[:], in_=wk_d.rearrange("p (o m) -> p o m", o=NCB)
            )
            nc.sync.dma_start(out=cst_sb[:], in_=cst_d[:])
            nc.scalar.dma_start(
                out=wq_sb[:], in_=wq_d.rearrange("p (o m) -> p o m", o=NCB)
            )
            xbv = xb_d.rearrange("(o p) n -> p o n", o=NCB)
            for t in range(4):
                csl = ds(t * 1024, 1024)
                eng = nc.sync if t % 2 == 0 else nc.scalar
                eng.dma_start(out=xb_sb[:, :, csl], in_=xbv[:, :, csl])
                if t == 1:
                    nc.sync.dma_start(
                        out=wv_sb[:], in_=wv_d.rearrange("p (o m) -> p o m", o=NCB)
                    )
            xpb_flat = xpb_sb.rearrange("p o c -> p (o c)")
            for cc in range(4):
                csl = ds(cc * 1024, 1024)
                eng = nc.scalar if cc % 2 == 0 else nc.sync
                eng.dma_start(out=xpb_flat[:, csl], in_=xpb_d[:, csl])

            nc.vector.memset(vT_sb[:, :, C:C + 1], 1.0)

            # ---- K projection, col-tiled (cb-outer so the four col-tiles
            # run concurrently per accumulation phase) ----
            def k_proj(qg):
                kp = pp_pj.tile([P, P], fp32, tag="pj", name=f"kp_{qg}")
                for cb in range(NCB):
                    for r in range(4):
                        j = 4 * qg + r
                        nc.tensor.matmul(
                            kp[32 * r:32 * r + 32, :],
                            wkT_sb[:, cb, :],
                            xb_sb[:, cb, ds(j * P, P)],
                            start=(cb == 0), stop=(cb == NCB - 1),
                            tile_position=(0, 32 * r),
                        )
                nc.vector.tensor_scalar_add(kf_sb[:, qg, :], kp, bk_sb)

            def v_proj_pair(t):
                # two key-blocks per PSUM tile; cast split ACT/DVE by parity
                vp = pp_pj.tile([P, 2, C], fp32, tag="pj", name=f"vp_{t}")
                for u in range(2):
                    j = 2 * t + u
                    nc.tensor.matmul(
                        vp[:, u], xb_sb[:, 0, ds(j * P, P)], wv_sb[:, 0],
                        start=True, stop=False,
                    )
                    nc.tensor.matmul(
                        vp[:, u], xb_sb[:, 1, ds(j * P, P)], wv_sb[:, 1],
                        start=False, stop=True,
                    )
                if t % 2 == 0:
                    nc.scalar.activation(
                        vT_sb[:, 2 * t:2 * t + 2, 0:C], vp,
                        mybir.ActivationFunctionType.Copy,
                    )
                else:
                    nc.vector.tensor_copy(vT_sb[:, 2 * t:2 * t + 2, 0:C], vp)

            # ---- Q projection chunk (replicated across the 4 row groups) ----
            def q_proj(t):
                pool, tag = qp_pool[0]
                qp = pool.tile([P, ST], fp32, tag=tag, name=f"qp_{t}")
                nc.tensor.matmul(
                    qp, wq_sb[:, 0], xb_sb[:, 0, ds(t * ST, ST)],
                    start=True, stop=False,
                )
                nc.tensor.matmul(
                    qp, wq_sb[:, 1], xb_sb[:, 1, ds(t * ST, ST)],
                    start=False, stop=True,
                )
                nc.vector.tensor_scalar_add(q_sb[:, ds(t * ST, ST)], qp, bq_sb)

            es_by_st = [[] for _ in range(NST)]
            mm_pool = [pp_mm_h]
            qp_pool = [(pp_pj, "pj")]

            def scores_group(st_i, g):
                # st0: pair-group g = blocks {2g, 2g+1} (head PSUM layout);
                # st1-3: triple g per GROUPS3.
                js = [2 * g, 2 * g + 1] if st_i == 0 else GROUPS3[g]
                nr = len(js)
                sps = mm_pool[0].tile(
                    [P, nr, ST], fp32, tag="mm", name=f"sps_{st_i}_{g}")
                for r, j in enumerate(js):
                    m = j % 4
                    nc.tensor.matmul(
                        sps[:, r],
                        kf_sb[32 * m:32 * (m + 1), j // 4, :],
                        q_sb[32 * m:32 * (m + 1), ds(st_i * ST, ST)],
                        start=True, stop=True,
                        tile_position=(32 * m, 0),
                    )
                e = epool.tile([P, nr, ST], bf16, name=f"e_{st_i}_{g}", tag="e")
                dve = g in (DVE_G0 if st_i == 0 else DVE_G)
                if dve:
                    nc.vector.tensor_scalar(
                        e.bitcast(i16)[:], sps[:], SCHRA_A, SCHRA_B,
                        mybir.AluOpType.mult, mybir.AluOpType.add,
                    )
                else:
                    nc.scalar.activation(
                        e[:], sps[:], mybir.ActivationFunctionType.Exp
                    )
                es_by_st[st_i].append(e)

            def epilogue(st_i, ib, out_ps):
                row = st_i * 4 + ib
                inv = ivpool.tile([P, 1], fp32)
                nc.vector.reciprocal(inv, out_ps[:, C:C + 1])
                nc.vector.tensor_scalar_mul(inv, inv, gm_sb)
                stg = stpool.tile([P, C], fp32)
                nc.vector.scalar_tensor_tensor(
                    stg, out_ps[:, 0:C], inv, xpb_sb[:, row, :],
                    op0=mybir.AluOpType.mult,
                    op1=mybir.AluOpType.add,
                )
                nc.sync.dma_start(out=yT_d[ds(row * P, P), :], in_=stg[:])

            def pv_mm(st_i, ib, j, out_ps):
                es = es_by_st[st_i]
                if st_i == 0:
                    e_ap = es[j // 2][:, j % 2, ds(ib * P, P)]
                else:
                    g = min(j // 3, 10)
                    e_ap = es[g][:, j - 3 * g, ds(ib * P, P)]
                nc.tensor.matmul(
                    out_ps, e_ap, vT_sb[:, j, :],
                    start=(j == 0), stop=(j == JB - 1),
                )

            # ---- st0 head, chunk-driven: 1024-col x chunk t enables k-proj
            # quads 2t/2t+1, score pair-groups 4t..4t+3, v-proj pairs
            # 4t..4t+3; the PV of ib0/ib1 chases the exp pipeline to keep
            # the PE stream dense (HAM full-clock hold).
            chase_j = [0]      # next j for the ib0/ib1 chase (synchronized)
            done_g = [0]

            def chase(max_j):
                while chase_j[0] < min(max_j, JB):
                    j = chase_j[0]
                    pv_mm(0, 0, j, out_a)
                    pv_mm(0, 1, j, out_b)
                    chase_j[0] += 1

            for t in range(4):
                k_proj(2 * t)
                if t == 0:
                    q_proj(0)
                k_proj(2 * t + 1)
                while done_g[0] < 4 * (t + 1):
                    scores_group(0, done_g[0])
                    done_g[0] += 1
                    v_proj_pair(done_g[0] - 1)
                    # hold the chase one group behind the exp pipeline so
                    # the in-order PE queue never blocks on a fresh exp
                    chase(2 * done_g[0] - 2)

            q_proj(1)
            chase(JB)
            epilogue(0, 0, out_a)
            epilogue(0, 1, out_b)

            # release the head PSUM layout; steady state: triple score
            # tiles (2x3 banks) + rotating PV out pool (2 banks).
            _free_b()
            _free_a()
            head_ps.close()
            with (
                tc.tile_pool(name="pp_mm", bufs=2, space="PSUM") as pp_mm,
                tc.tile_pool(name="pp_out", bufs=2, space="PSUM") as pp_out,
            ):
                mm_pool[0] = pp_mm
                qp_pool[0] = (pp_out, "out")

                nxt = [0]
                cnt = [0]

                def weave(st_n, every):
                    cnt[0] += 1
                    if cnt[0] % every == 0 and st_n < NST and nxt[0] < NGR:
                        scores_group(st_n, nxt[0])
                        nxt[0] += 1

                # st0 ib2/ib3 run dense from the stored e-tiles, weaving
                # st1's triples (11 groups over 64 matmuls).
                for ib in (2, 3):
                    out_ps = pp_out.tile(
                        [P, C + 1], fp32, tag="out", name=f"out_0_{ib}")
                    for j in range(JB):
                        pv_mm(0, ib, j, out_ps)
                        weave(1, 5)
                    epilogue(0, ib, out_ps)
                while nxt[0] < NGR:
                    scores_group(1, nxt[0])
                    nxt[0] += 1

                # ---- st1..3 PV; next supertile's scores/exp woven in ----
                for st_i in range(1, NST):
                    if st_i + 1 < NST:
                        q_proj(st_i + 1)
                    nxt[0] = 0
                    cnt[0] = 0
                    for ib in range(4):
                        out_ps = pp_out.tile(
                            [P, C + 1], fp32, tag="out", name=f"out_{st_i}_{ib}")
                        for j in range(JB):
                            pv_mm(st_i, ib, j, out_ps)
                            weave(st_i + 1, 10)
                        epilogue(st_i, ib, out_ps)
                    while st_i + 1 < NST and nxt[0] < NGR:
                        scores_group(st_i + 1, nxt[0])
                        nxt[0] += 1

    return nc


def _get_program():
    global _PROG
    if _PROG is None:
        _PROG = _build_program()
        if not _PROG.is_finalized():
            _PROG.finalize()
    return _PROG


def kernel(x, Wq, bq, Wk, bk, Wv, bv, gamma):
    global LAST_RESULT
    import ml_dtypes
    from concourse.bass_utils import run_bass_kernel_spmd

    bf16 = ml_dtypes.bfloat16
    x = np.ascontiguousarray(np.asarray(x, dtype=np.float32))
    Wq = np.asarray(Wq, dtype=np.float32)
    bq = np.asarray(bq, dtype=np.float32)
    Wk = np.asarray(Wk, dtype=np.float32)
    bk = np.asarray(bk, dtype=np.float32)
    Wv = np.asarray(Wv, dtype=np.float32)
    bv = np.asarray(bv, dtype=np.float32)
    gamma = np.asarray(gamma, dtype=np.float32)

    # wq replicated into all four 32-row groups of the PE array
    wq_rep = np.zeros((C, P), dtype=np.float32)
    for r in range(4):
        wq_rep[:, 32 * r:32 * (r + 1)] = Wq.T
    gval = float(gamma.reshape(-1)[0])
    consts = np.zeros((P, 4), dtype=np.float32)
    consts[:, 0] = np.tile(bq, 4)
    consts[:, 1] = np.tile(bk, 4)
    consts[:, 2] = gval

    def _swz(a):
        # [C, F] -> [128, NCB*F]: exact SBUF layout (partition-major)
        f = a.reshape(NCB, P, -1)
        return np.ascontiguousarray(
            f.transpose(1, 0, 2).reshape(P, -1).astype(bf16)
        )

    wq_pre = _swz(wq_rep)
    wkT_pre = _swz(Wk.T)
    wv_pre = _swz(Wv.T)

    xf = x.reshape(B, C, N)
    in_maps = []
    for core in range(8):
        b, h = core // 2, core % 2
        xb = xf[b]
        if h == 0:
            x_roll = xb
        else:
            x_roll = np.concatenate([xb[:, NH:], xb[:, :NH]], axis=1)
        # xpb[p, o, c] = x_roll[c, o*128 + p] + gamma*bv[c]  (SBUF layout)
        xqT = x_roll[:, :NH].T + gval * bv[None, :]
        xpb = np.ascontiguousarray(
            xqT.reshape(NH // P, P, C).transpose(1, 0, 2).reshape(P, (NH // P) * C)
        ).astype(np.float32)
        in_maps.append({
            "xb": np.ascontiguousarray(x_roll.astype(bf16)),
            "xpb": xpb,
            "wq_pre": wq_pre,
            "wkT_pre": wkT_pre,
            "wv_pre": wv_pre,
            "consts": consts,
        })

    nc = _get_program()
    res = run_bass_kernel_spmd(
        nc, in_maps, core_ids=list(range(8)),
        trace=bool(os.environ.get("BASS_TRACE")),
    )
    LAST_RESULT = res

    out = np.empty((B, C, N), dtype=np.float32)
    for core in range(8):
        b, h = core // 2, core % 2
        yT = res.results[core]["yT"]
        out[b][:, h * NH:(h + 1) * NH] = yT.T
    return out.reshape(B, C, H, W)


# revision 25
# speedup vs baseline: 1.0897x; 1.0897x over previous
"""Position-attention kernel for Trainium2 (8 NeuronCores, SPMD).

Reference computation (per batch b):
    q = Wq @ x + bq        [32, 4096]
    k = Wk @ x + bk        [32, 4096]
    v = Wv @ x + bv        [256, 4096]
    attn = softmax_j(q_i . k_j)           [4096, 4096]
    out[c, i] = sum_j v[c, j] attn[i, j]
    y = gamma * out + x

Sharding: B=4 batches x 2 query-halves -> 8 cores. Each core computes the
full softmax rows for its 2048 queries against all 4096 keys of its batch.
Host rotates x columns per core so the core's query half is always columns
0:2048 (softmax and the PV contraction are invariant to key/value column
order, as long as K and V use the same order).

Device-side structure (per core):
  - projections in bf16 (x pre-cast on host; weights pre-packed on host).
  - K projection col-tiled: stationary WkT [128, 32] at tile_position
    (0, 32*(j%4)) so four 32-row key-block outputs land packed in one PSUM
    tile; kf stored [32*(j%4)+d, j//4, :].
  - scores computed transposed (sT[j, i]) in PSUM, 2 key-blocks at a time
    packed into PE row-groups 32*(j%4) via tile_position (the K=32
    contractions run concurrently); q replicated into all four row groups.
  - exp is split across two engines: ACT does half the groups (exact exp),
    DVE the other half with a Schraudolph fast-exp (one tensor_scalar:
    int16 bits = round(s*A + B), bitcast to bf16; +-3% sawtooth that
    largely cancels in the softmax ratio). Removes ACT as co-bottleneck.
  - PV: out[i, c] = sum_j e[j, i] * vT[j, c] with e-blocks stationary;
    vT carries an all-ones column so column 256 of the output is the
    softmax denominator (per-partition = per-query).
  - head is chunk-driven (one 512-col x DMA per k-proj quad) and the PV of
    st0's first two row-blocks CHASES the exp pipeline so the PE stream
    never breaks: the HAM activity monitor grants full clock only under
    sustained dense matmul activity, and a failed window costs a long
    half-clock lockout.
  - epilogue: y_T[i, :] = out[i, :] * (gamma / sum_i) + (x_T[i, :] +
    gamma * bv); bv-folding works because sum_j attn = 1. Output written
    transposed; host transposes back (pure layout).
"""

import os
import numpy as np

P = 128
B = 4
C = 256
CQ = 32
H = W = 64
N = H * W            # 4096 keys per batch
NH = N // 2          # 2048 queries per core
NCB = C // P         # 2 channel blocks
ST = 512             # query supertile
NST = NH // ST       # 4
JB = N // P          # 32 key blocks
NQG = JB // 4        # 8 k-proj quad groups
NGR0 = JB // 2       # 16 score pair-groups for st0 (2-bank PSUM tiles)
# st1-3 use triples (3-bank tiles, fewer per-burst overheads)
GROUPS3 = [list(range(3 * g, min(3 * g + 3, 32))) for g in range(11)]
NGR = len(GROUPS3)

# exp engine assignment: which groups go to the DVE (Schraudolph).
# st0's DVE is also busy with v-casts and bias adds, so it gets fewer,
# spread out so the ib0/ib1 chase is never starved.
DVE_G0 = {2, 5, 8, 10, 12, 14}
DVE_G = {2, 4, 6, 8}

# Schraudolph fast-exp constants: bf16 bits = round(s * SCHRA_A + SCHRA_B)
SCHRA_A = 184.66496523378732          # 128 * log2(e)
SCHRA_B = 16250.515                    # centered: 127*128 - 128*0.0602/(2 ln2)

_PROG = None         # cached build
LAST_RESULT = None   # BassKernelResults of the last run (for test harness)


def _build_program():
    import concourse.mybir as mybir
    import concourse.tile as tile
    from concourse import bacc
    from concourse.bass import ds

    fp32 = mybir.dt.float32
    bf16 = mybir.dt.bfloat16
    i16 = mybir.dt.int16

    nc = bacc.Bacc(None, target_bir_lowering=False, debug=False)

    xb_d = nc.declare_dram_parameter("xb", [C, N], bf16, isOutput=False)
    # xpb = xT + gamma*bv, already in SBUF layout [p, row_block*C]
    xpb_d = nc.declare_dram_parameter("xpb", [P, (NH // P) * C], fp32, isOutput=False)
    wq_d = nc.declare_dram_parameter("wq_pre", [P, NCB * P], bf16, isOutput=False)
    wk_d = nc.declare_dram_parameter("wkT_pre", [P, NCB * CQ], bf16, isOutput=False)
    wv_d = nc.declare_dram_parameter("wv_pre", [P, NCB * C], bf16, isOutput=False)
    # consts columns: 0=bq_rep, 1=bk_pack, 2=gamma, 3=pad (one DMA trigger)
    cst_d = nc.declare_dram_parameter("consts", [P, 4], fp32, isOutput=False)
    yT_d = nc.declare_dram_parameter("yT", [NH, C], fp32, isOutput=True)

    from contextlib import ExitStack

    with tile.TileContext(nc) as tc:
        with (
            tc.tile_pool(name="singles", bufs=1) as singles,
            tc.tile_pool(name="epool", bufs=30) as epool,
            tc.tile_pool(name="stpool", bufs=4) as stpool,
            tc.tile_pool(name="ivpool", bufs=4) as ivpool,
        ):
            # PSUM, head phase (8 banks): score pair tiles 2x2, proj pool 2,
            # and two dedicated banks for the st0 ib0/ib1 PV accumulators.
            # After the head these are all released and replaced by the
            # steady-state layout: score triple tiles 2x3 + rotating out 2.
            head_ps = ExitStack()
            pp_mm_h = head_ps.enter_context(
                tc.tile_pool(name="pp_mm_h", bufs=2, space="PSUM"))
            pp_pj = head_ps.enter_context(
                tc.tile_pool(name="pp_pj", bufs=2, space="PSUM"))
            out_a, _free_a = tc.tile([P, C + 1], fp32, space="PSUM", name="out_a")
            out_b, _free_b = tc.tile([P, C + 1], fp32, space="PSUM", name="out_b")
            outs = [out_a, out_b]

            # ---- persistent SBUF tensors ----
            xb_sb = singles.tile([P, NCB, N], bf16)
            xpb_sb = singles.tile([P, NH // P, C], fp32)  # xT + gamma*bv
            wq_sb = singles.tile([P, NCB, P], bf16)
            wkT_sb = singles.tile([P, NCB, CQ], bf16)
            wv_sb = singles.tile([P, NCB, C], bf16)
            cst_sb = singles.tile([P, 4], fp32)
            bq_sb = cst_sb[:, 0:1]
            bk_sb = cst_sb[:, 1:2]
            gm_sb = cst_sb[:, 2:3]
            kf_sb = singles.tile([P, NQG, P], bf16)  # row 32*(j%4)+d, quad j//4
            q_sb = singles.tile([P, NH], bf16)       # q replicated in 4 groups
            vT_sb = singles.tile([P, JB, C + 1], bf16)  # col C is all-ones

            # dummy exp pre-loads the ACT function table (~2.7us) so the
            # first real exp doesn't pay it. No PE warmup: tripping the HAM
            # before activity can be SUSTAINED causes a half-clock lockout.
            warm_e = singles.tile([1, 1], fp32)
            nc.vector.memset(warm_e[:], 0.0)
            nc.scalar.activation(
                warm_e, warm_e, mybir.ActivationFunctionType.Exp
            )

            # ---- input DMAs. Each dma_start trigger occupies its queue for
            # ~600ns and transfers on one HWDGE ring serialize, so use FEW,
            # LARGE transfers (>=2KB per partition line for full SDMA
            # efficiency) spread over BOTH HWDGE rings (sync + scalar): xb
            # in four 1024-col chunks, alternating rings. xpb (residual)
            # last: first use is the st0/ib0 epilogue.
            nc.sync.dma_start(
                out=wkT_sb[:], in_=wk_d.rearrange("p (o m) -> p o m", o=NCB)
            )
            nc.sync.dma_start(out=cst_sb[:], in_=cst_d[:])
            nc.scalar.dma_start(
                out=wq_sb[:], in_=wq_d.rearrange("p (o m) -> p o m", o=NCB)
            )
            xbv = xb_d.rearrange("(o p) n -> p o n", o=NCB)
            xb_spans = [(0, 512, nc.sync), (512, 512, nc.sync),
                        (1024, 1024, nc.scalar), (2048, 1024, nc.sync),
                        (3072, 1024, nc.scalar)]
            for c0, cw, eng in xb_spans:
                csl = ds(c0, cw)
                eng.dma_start(out=xb_sb[:, :, csl], in_=xbv[:, :, csl])
                if c0 == 512:
                    nc.scalar.dma_start(
                        out=wv_sb[:], in_=wv_d.rearrange("p (o m) -> p o m", o=NCB)
                    )
            xpb_flat = xpb_sb.rearrange("p o c -> p (o c)")
            for cc in range(4):
                csl = ds(cc * 1024, 1024)
                eng = nc.scalar if cc % 2 == 0 else nc.sync
                eng.dma_start(out=xpb_flat[:, csl], in_=xpb_d[:, csl])

            nc.vector.memset(vT_sb[:, :, C:C + 1], 1.0)

            # ---- K projection, col-tiled (cb-outer so the four col-tiles
            # run concurrently per accumulation phase) ----
            def k_proj(qg):
                kp = pp_pj.tile([P, P], fp32, tag="pj", name=f"kp_{qg}")
                for cb in range(NCB):
                    for r in range(4):
                        j = 4 * qg + r
                        nc.tensor.matmul(
                            kp[32 * r:32 * r + 32, :],
                            wkT_sb[:, cb, :],
                            xb_sb[:, cb, ds(j * P, P)],
                            start=(cb == 0), stop=(cb == NCB - 1),
                            tile_position=(0, 32 * r),
                        )
                nc.vector.tensor_scalar_add(kf_sb[:, qg, :], kp, bk_sb)

            def v_proj_pair(t):
                # two key-blocks per PSUM tile; cast split ACT/DVE by parity
                vp = pp_pj.tile([P, 2, C], fp32, tag="pj", name=f"vp_{t}")
                for u in range(2):
                    j = 2 * t + u
                    nc.tensor.matmul(
                        vp[:, u], xb_sb[:, 0, ds(j * P, P)], wv_sb[:, 0],
                        start=True, stop=False,
                    )
                    nc.tensor.matmul(
                        vp[:, u], xb_sb[:, 1, ds(j * P, P)], wv_sb[:, 1],
                        start=False, stop=True,
                    )
                if t % 2 == 0:
                    nc.scalar.activation(
                        vT_sb[:, 2 * t:2 * t + 2, 0:C], vp,
                        mybir.ActivationFunctionType.Copy,
                    )
                else:
                    nc.vector.tensor_copy(vT_sb[:, 2 * t:2 * t + 2, 0:C], vp)

            # ---- Q projection chunk (replicated across the 4 row groups) ----
            def q_proj(t):
                pool, tag = qp_pool[0]
                qp = pool.tile([P, ST], fp32, tag=tag, name=f"qp_{t}")
                nc.tensor.matmul(
                    qp, wq_sb[:, 0], xb_sb[:, 0, ds(t * ST, ST)],
                    start=True, stop=False,
                )
                nc.tensor.matmul(
                    qp, wq_sb[:, 1], xb_sb[:, 1, ds(t * ST, ST)],
                    start=False, stop=True,
                )
                nc.vector.tensor_scalar_add(q_sb[:, ds(t * ST, ST)], qp, bq_sb)

            es_by_st = [[] for _ in range(NST)]
            mm_pool = [pp_mm_h]
            qp_pool = [(pp_pj, "pj")]

            def scores_group(st_i, g):
                # st0: pair-group g = blocks {2g, 2g+1} (head PSUM layout);
                # st1-3: triple g per GROUPS3.
                js = [2 * g, 2 * g + 1] if st_i == 0 else GROUPS3[g]
                nr = len(js)
                sps = mm_pool[0].tile(
                    [P, nr, ST], fp32, tag="mm", name=f"sps_{st_i}_{g}")
                for r, j in enumerate(js):
                    m = j % 4
                    nc.tensor.matmul(
                        sps[:, r],
                        kf_sb[32 * m:32 * (m + 1), j // 4, :],
                        q_sb[32 * m:32 * (m + 1), ds(st_i * ST, ST)],
                        start=True, stop=True,
                        tile_position=(32 * m, 0),
                    )
                e = epool.tile([P, nr, ST], bf16, name=f"e_{st_i}_{g}", tag="e")
                dve = g in (DVE_G0 if st_i == 0 else DVE_G)
                if dve:
                    nc.vector.tensor_scalar(
                        e.bitcast(i16)[:], sps[:], SCHRA_A, SCHRA_B,
                        mybir.AluOpType.mult, mybir.AluOpType.add,
                    )
                else:
                    nc.scalar.activation(
                        e[:], sps[:], mybir.ActivationFunctionType.Exp
                    )
                es_by_st[st_i].append(e)

            def epilogue(st_i, ib, out_ps):
                row = st_i * 4 + ib
                inv = ivpool.tile([P, 1], fp32)
                nc.vector.reciprocal(inv, out_ps[:, C:C + 1])
                nc.vector.tensor_scalar_mul(inv, inv, gm_sb)
                stg = stpool.tile([P, C], fp32)
                nc.vector.scalar_tensor_tensor(
                    stg, out_ps[:, 0:C], inv, xpb_sb[:, row, :],
                    op0=mybir.AluOpType.mult,
                    op1=mybir.AluOpType.add,
                )
                nc.sync.dma_start(out=yT_d[ds(row * P, P), :], in_=stg[:])

            def pv_mm(st_i, ib, j, out_ps):
                es = es_by_st[st_i]
                if st_i == 0:
                    e_ap = es[j // 2][:, j % 2, ds(ib * P, P)]
                else:
                    g = min(j // 3, 10)
                    e_ap = es[g][:, j - 3 * g, ds(ib * P, P)]
                nc.tensor.matmul(
                    out_ps, e_ap, vT_sb[:, j, :],
                    start=(j == 0), stop=(j == JB - 1),
                )

            # ---- st0 head, chunk-driven: 1024-col x chunk t enables k-proj
            # quads 2t/2t+1, score pair-groups 4t..4t+3, v-proj pairs
            # 4t..4t+3; the PV of ib0/ib1 chases the exp pipeline to keep
            # the PE stream dense (HAM full-clock hold).
            chase_j = [0]      # next j for the ib0/ib1 chase (synchronized)
            done_g = [0]

            def chase(max_j):
                while chase_j[0] < min(max_j, JB):
                    j = chase_j[0]
                    pv_mm(0, 0, j, out_a)
                    pv_mm(0, 1, j, out_b)
                    chase_j[0] += 1

            for t in range(4):
                k_proj(2 * t)
                if t == 0:
                    q_proj(0)
                k_proj(2 * t + 1)
                while done_g[0] < 4 * (t + 1):
                    scores_group(0, done_g[0])
                    done_g[0] += 1
                    v_proj_pair(done_g[0] - 1)
                    # hold the chase one group behind the exp pipeline so
                    # the in-order PE queue never blocks on a fresh exp
                    chase(2 * done_g[0] - 2)

            # all remaining q chunks now (xb fully resident): keeps the q
            # bias-adds off the congested supertile boundaries
            q_proj(1)
            q_proj(2)
            q_proj(3)
            chase(JB)
            epilogue(0, 0, out_a)
            epilogue(0, 1, out_b)

            # release the head PSUM layout; steady state: triple score
            # tiles (2x3 banks) + rotating PV out pool (2 banks).
            _free_b()
            _free_a()
            head_ps.close()
            with (
                tc.tile_pool(name="pp_mm", bufs=2, space="PSUM") as pp_mm,
                tc.tile_pool(name="pp_out", bufs=2, space="PSUM") as pp_out,
            ):
                mm_pool[0] = pp_mm
                qp_pool[0] = (pp_out, "out")

                nxt = [0]
                cnt = [0]

                def weave(st_n, every):
                    cnt[0] += 1
                    if cnt[0] % every == 0 and st_n < NST and nxt[0] < NGR:
                        scores_group(st_n, nxt[0])
                        nxt[0] += 1

                # st0 ib2/ib3 run dense from the stored e-tiles, weaving
                # st1's triples (11 groups over 64 matmuls).
                for ib in (2, 3):
                    out_ps = pp_out.tile(
                        [P, C + 1], fp32, tag="out", name=f"out_0_{ib}")
                    for j in range(JB):
                        pv_mm(0, ib, j, out_ps)
                        weave(1, 5)
                    epilogue(0, ib, out_ps)
                while nxt[0] < NGR:
                    scores_group(1, nxt[0])
                    nxt[0] += 1

                # ---- st1..3 PV; next supertile's scores/exp woven in ----
                for st_i in range(1, NST):
                    nxt[0] = 0
                    cnt[0] = 0
                    for ib in range(4):
                        out_ps = pp_out.tile(
                            [P, C + 1], fp32, tag="out", name=f"out_{st_i}_{ib}")
                        for j in range(JB):
                            pv_mm(st_i, ib, j, out_ps)
                            weave(st_i + 1, 10)
                        epilogue(st_i, ib, out_ps)
                    while st_i + 1 < NST and nxt[0] < NGR:
                        scores_group(st_i + 1, nxt[0])
                        nxt[0] += 1

    return nc


def _get_program():
    global _PROG
    if _PROG is None:
        _PROG = _build_program()
        if not _PROG.is_finalized():
            _PROG.finalize()
    return _PROG


def kernel(x, Wq, bq, Wk, bk, Wv, bv, gamma):
    global LAST_RESULT
    import ml_dtypes
    from concourse.bass_utils import run_bass_kernel_spmd

    bf16 = ml_dtypes.bfloat16
    x = np.ascontiguousarray(np.asarray(x, dtype=np.float32))
    Wq = np.asarray(Wq, dtype=np.float32)
    bq = np.asarray(bq, dtype=np.float32)
    Wk = np.asarray(Wk, dtype=np.float32)
    bk = np.asarray(bk, dtype=np.float32)
    Wv = np.asarray(Wv, dtype=np.float32)
    bv = np.asarray(bv, dtype=np.float32)
    gamma = np.asarray(gamma, dtype=np.float32)

    # wq replicated into all four 32-row groups of the PE array
    wq_rep = np.zeros((C, P), dtype=np.float32)
    for r in range(4):
        wq_rep[:, 32 * r:32 * (r + 1)] = Wq.T
    gval = float(gamma.reshape(-1)[0])
    consts = np.zeros((P, 4), dtype=np.float32)
    consts[:, 0] = np.tile(bq, 4)
    consts[:, 1] = np.tile(bk, 4)
    consts[:, 2] = gval

    def _swz(a):
        # [C, F] -> [128, NCB*F]: exact SBUF layout (partition-major)
        f = a.reshape(NCB, P, -1)
        return np.ascontiguousarray(
            f.transpose(1, 0, 2).reshape(P, -1).astype(bf16)
        )

    wq_pre = _swz(wq_rep)
    wkT_pre = _swz(Wk.T)
    wv_pre = _swz(Wv.T)

    xf = x.reshape(B, C, N)
    in_maps = []
    for core in range(8):
        b, h = core // 2, core % 2
        xb = xf[b]
        if h == 0:
            x_roll = xb
        else:
            x_roll = np.concatenate([xb[:, NH:], xb[:, :NH]], axis=1)
        # xpb[p, o, c] = x_roll[c, o*128 + p] + gamma*bv[c]  (SBUF layout)
        xqT = x_roll[:, :NH].T + gval * bv[None, :]
        xpb = np.ascontiguousarray(
            xqT.reshape(NH // P, P, C).transpose(1, 0, 2).reshape(P, (NH // P) * C)
        ).astype(np.float32)
        in_maps.append({
            "xb": np.ascontiguousarray(x_roll.astype(bf16)),
            "xpb": xpb,
            "wq_pre": wq_pre,
            "wkT_pre": wkT_pre,
            "wv_pre": wv_pre,
            "consts": consts,
        })

    nc = _get_program()
    res = run_bass_kernel_spmd(
        nc, in_maps, core_ids=list(range(8)),
        trace=bool(os.environ.get("BASS_TRACE")),
    )
    LAST_RESULT = res

    out = np.empty((B, C, N), dtype=np.float32)
    for core in range(8):
        b, h = core // 2, core % 2
        yT = res.results[core]["yT"]
        out[b][:, h * NH:(h + 1) * NH] = yT.T
    return out.reshape(B, C, H, W)


# revision 27
# speedup vs baseline: 1.1067x; 1.0155x over previous
"""Position-attention kernel for Trainium2 (8 NeuronCores, SPMD).

Reference computation (per batch b):
    q = Wq @ x + bq        [32, 4096]
    k = Wk @ x + bk        [32, 4096]
    v = Wv @ x + bv        [256, 4096]
    attn = softmax_j(q_i . k_j)           [4096, 4096]
    out[c, i] = sum_j v[c, j] attn[i, j]
    y = gamma * out + x

Sharding: B=4 batches x 2 query-halves -> 8 cores. Each core computes the
full softmax rows for its 2048 queries against all 4096 keys of its batch.
Host rotates x columns per core so the core's query half is always columns
0:2048 (softmax and the PV contraction are invariant to key/value column
order, as long as K and V use the same order).

Device-side structure (per core):
  - projections in bf16 (x pre-cast on host; weights pre-packed on host).
  - K projection col-tiled: stationary WkT [128, 32] at tile_position
    (0, 32*(j%4)) so four 32-row key-block outputs land packed in one PSUM
    tile; kf stored [32*(j%4)+d, j//4, :].
  - scores computed transposed (sT[j, i]) in PSUM, 2 key-blocks at a time
    packed into PE row-groups 32*(j%4) via tile_position (the K=32
    contractions run concurrently); q replicated into all four row groups.
  - exp is split across two engines: ACT does half the groups (exact exp),
    DVE the other half with a Schraudolph fast-exp (one tensor_scalar:
    int16 bits = round(s*A + B), bitcast to bf16; +-3% sawtooth that
    largely cancels in the softmax ratio). Removes ACT as co-bottleneck.
  - PV: out[i, c] = sum_j e[j, i] * vT[j, c] with e-blocks stationary;
    vT carries an all-ones column so column 256 of the output is the
    softmax denominator (per-partition = per-query).
  - head is chunk-driven (one 512-col x DMA per k-proj quad) and the PV of
    st0's first two row-blocks CHASES the exp pipeline so the PE stream
    never breaks: the HAM activity monitor grants full clock only under
    sustained dense matmul activity, and a failed window costs a long
    half-clock lockout.
  - epilogue: y_T[i, :] = out[i, :] * (gamma / sum_i) + (x_T[i, :] +
    gamma * bv); bv-folding works because sum_j attn = 1. Output written
    transposed; host transposes back (pure layout).
"""

import os
import numpy as np

P = 128
B = 4
C = 256
CQ = 32
H = W = 64
N = H * W            # 4096 keys per batch
NH = N // 2          # 2048 queries per core
NCB = C // P         # 2 channel blocks
ST = 512             # query supertile
NST = NH // ST       # 4
JB = N // P          # 32 key blocks
NQG = JB // 4        # 8 k-proj quad groups
NGR0 = JB // 2       # 16 score pair-groups for st0 (2-bank PSUM tiles)
# st1-3 use triples (3-bank tiles, fewer per-burst overheads)
GROUPS3 = [list(range(3 * g, min(3 * g + 3, 32))) for g in range(11)]
NGR = len(GROUPS3)

# exp engine assignment: which groups go to the DVE (Schraudolph).
# st0's DVE is also busy with v-casts and bias adds, so it gets fewer,
# spread out so the ib0/ib1 chase is never starved.
DVE_G0 = {2, 5, 8, 11, 14}
DVE_G = {2, 4, 6, 8}

# Schraudolph fast-exp constants: bf16 bits = round(s * SCHRA_A + SCHRA_B)
SCHRA_A = 184.66496523378732          # 128 * log2(e)
SCHRA_B = 16250.515                    # centered: 127*128 - 128*0.0602/(2 ln2)

_PROG = None         # cached build
LAST_RESULT = None   # BassKernelResults of the last run (for test harness)


def _build_program():
    import concourse.mybir as mybir
    import concourse.tile as tile
    from concourse import bacc
    from concourse.bass import ds

    fp32 = mybir.dt.float32
    bf16 = mybir.dt.bfloat16
    i16 = mybir.dt.int16

    nc = bacc.Bacc(None, target_bir_lowering=False, debug=False)

    xb_d = nc.declare_dram_parameter("xb", [C, N], bf16, isOutput=False)
    # xpb = xT + gamma*bv, already in SBUF layout [p, row_block*C]
    xpb_d = nc.declare_dram_parameter("xpb", [P, (NH // P) * C], fp32, isOutput=False)
    wq_d = nc.declare_dram_parameter("wq_pre", [P, NCB * P], bf16, isOutput=False)
    wk_d = nc.declare_dram_parameter("wkT_pre", [P, NCB * CQ], bf16, isOutput=False)
    wv_d = nc.declare_dram_parameter("wv_pre", [P, NCB * C], bf16, isOutput=False)
    # consts columns: 0=bq_rep, 1=bk_pack, 2=gamma, 3=pad (one DMA trigger)
    cst_d = nc.declare_dram_parameter("consts", [P, 4], fp32, isOutput=False)
    yT_d = nc.declare_dram_parameter("yT", [NH, C], fp32, isOutput=True)

    from contextlib import ExitStack

    with tile.TileContext(nc) as tc:
        with (
            tc.tile_pool(name="singles", bufs=1) as singles,
            tc.tile_pool(name="epool", bufs=30) as epool,
            tc.tile_pool(name="stpool", bufs=4) as stpool,
            tc.tile_pool(name="ivpool", bufs=4) as ivpool,
        ):
            # PSUM, head phase (8 banks): score pair tiles 2x2, proj pool 2,
            # and two dedicated banks for the st0 ib0/ib1 PV accumulators.
            # After the head these are all released and replaced by the
            # steady-state layout: score triple tiles 2x3 + rotating out 2.
            head_ps = ExitStack()
            pp_mm_h = head_ps.enter_context(
                tc.tile_pool(name="pp_mm_h", bufs=2, space="PSUM"))
            pp_pj = head_ps.enter_context(
                tc.tile_pool(name="pp_pj", bufs=2, space="PSUM"))
            out_a, _free_a = tc.tile([P, C + 1], fp32, space="PSUM", name="out_a")
            out_b, _free_b = tc.tile([P, C + 1], fp32, space="PSUM", name="out_b")
            outs = [out_a, out_b]

            # ---- persistent SBUF tensors ----
            xb_sb = singles.tile([P, NCB, N], bf16)
            xpb_sb = singles.tile([P, NH // P, C], fp32)  # xT + gamma*bv
            wq_sb = singles.tile([P, NCB, P], bf16)
            wkT_sb = singles.tile([P, NCB, CQ], bf16)
            wv_sb = singles.tile([P, NCB, C], bf16)
            cst_sb = singles.tile([P, 4], fp32)
            bq_sb = cst_sb[:, 0:1]
            bk_sb = cst_sb[:, 1:2]
            gm_sb = cst_sb[:, 2:3]
            kf_sb = singles.tile([P, NQG, P], bf16)  # row 32*(j%4)+d, quad j//4
            q_sb = singles.tile([P, NH], bf16)       # q replicated in 4 groups
            vT_sb = singles.tile([P, JB, C + 1], bf16)  # col C is all-ones

            # dummy exp pre-loads the ACT function table (~2.7us) so the
            # first real exp doesn't pay it. No PE warmup: tripping the HAM
            # before activity can be SUSTAINED causes a half-clock lockout.
            warm_e = singles.tile([1, 1], fp32)
            nc.vector.memset(warm_e[:], 0.0)
            nc.scalar.activation(
                warm_e, warm_e, mybir.ActivationFunctionType.Exp
            )

            # ---- input DMAs. Each dma_start trigger occupies its queue for
            # ~600ns and transfers on one HWDGE ring serialize, so use FEW,
            # LARGE transfers (>=2KB per partition line for full SDMA
            # efficiency) spread over BOTH HWDGE rings (sync + scalar): xb
            # in four 1024-col chunks, alternating rings. xpb (residual)
            # last: first use is the st0/ib0 epilogue.
            xbv = xb_d.rearrange("(o p) n -> p o n", o=NCB)
            nc.sync.dma_start(
                out=wkT_sb[:], in_=wk_d.rearrange("p (o m) -> p o m", o=NCB)
            )
            nc.scalar.dma_start(
                out=wq_sb[:], in_=wq_d.rearrange("p (o m) -> p o m", o=NCB)
            )
            xb_spans = [(0, 512, nc.sync), (1024, 1024, nc.scalar),
                        (512, 512, nc.sync), (2048, 1024, nc.sync),
                        (3072, 1024, nc.scalar)]
            for idx, (c0, cw, eng) in enumerate(xb_spans):
                csl = ds(c0, cw)
                eng.dma_start(out=xb_sb[:, :, csl], in_=xbv[:, :, csl])
                if idx == 0:
                    nc.sync.dma_start(out=cst_sb[:], in_=cst_d[:])
                elif idx == 1:
                    nc.scalar.dma_start(
                        out=wv_sb[:], in_=wv_d.rearrange("p (o m) -> p o m", o=NCB)
                    )
            # xpb all on the sync ring: the scalar ring's queue is the ACT
            # engine, which needs to start the exp stream ASAP
            xpb_flat = xpb_sb.rearrange("p o c -> p (o c)")
            for cc in range(4):
                csl = ds(cc * 1024, 1024)
                nc.sync.dma_start(out=xpb_flat[:, csl], in_=xpb_d[:, csl])

            nc.vector.memset(vT_sb[:, :, C:C + 1], 1.0)

            # ---- K projection, col-tiled (cb-outer so the four col-tiles
            # run concurrently per accumulation phase) ----
            def k_proj(qg):
                kp = pp_pj.tile([P, P], fp32, tag="pj", name=f"kp_{qg}")
                for cb in range(NCB):
                    for r in range(4):
                        j = 4 * qg + r
                        nc.tensor.matmul(
                            kp[32 * r:32 * r + 32, :],
                            wkT_sb[:, cb, :],
                            xb_sb[:, cb, ds(j * P, P)],
                            start=(cb == 0), stop=(cb == NCB - 1),
                            tile_position=(0, 32 * r),
                        )
                nc.vector.tensor_scalar_add(kf_sb[:, qg, :], kp, bk_sb)

            def v_proj_pair(t):
                # two key-blocks per PSUM tile; cast split ACT/DVE by parity
                vp = pp_pj.tile([P, 2, C], fp32, tag="pj", name=f"vp_{t}")
                for u in range(2):
                    j = 2 * t + u
                    nc.tensor.matmul(
                        vp[:, u], xb_sb[:, 0, ds(j * P, P)], wv_sb[:, 0],
                        start=True, stop=False,
                    )
                    nc.tensor.matmul(
                        vp[:, u], xb_sb[:, 1, ds(j * P, P)], wv_sb[:, 1],
                        start=False, stop=True,
                    )
                if t % 2 == 0:
                    nc.scalar.activation(
                        vT_sb[:, 2 * t:2 * t + 2, 0:C], vp,
                        mybir.ActivationFunctionType.Copy,
                    )
                else:
                    nc.vector.tensor_copy(vT_sb[:, 2 * t:2 * t + 2, 0:C], vp)

            # ---- Q projection chunk (replicated across the 4 row groups) ----
            def q_proj(t):
                pool, tag = qp_pool[0]
                qp = pool.tile([P, ST], fp32, tag=tag, name=f"qp_{t}")
                nc.tensor.matmul(
                    qp, wq_sb[:, 0], xb_sb[:, 0, ds(t * ST, ST)],
                    start=True, stop=False,
                )
                nc.tensor.matmul(
                    qp, wq_sb[:, 1], xb_sb[:, 1, ds(t * ST, ST)],
                    start=False, stop=True,
                )
                nc.vector.tensor_scalar_add(q_sb[:, ds(t * ST, ST)], qp, bq_sb)

            es_by_st = [[] for _ in range(NST)]
            mm_pool = [pp_mm_h]
            qp_pool = [(pp_pj, "pj")]

            def scores_group(st_i, g):
                # st0: pair-group g = blocks {2g, 2g+1} (head PSUM layout);
                # st1-3: triple g per GROUPS3.
                js = [2 * g, 2 * g + 1] if st_i == 0 else GROUPS3[g]
                nr = len(js)
                sps = mm_pool[0].tile(
                    [P, nr, ST], fp32, tag="mm", name=f"sps_{st_i}_{g}")
                for r, j in enumerate(js):
                    m = j % 4
                    nc.tensor.matmul(
                        sps[:, r],
                        kf_sb[32 * m:32 * (m + 1), j // 4, :],
                        q_sb[32 * m:32 * (m + 1), ds(st_i * ST, ST)],
                        start=True, stop=True,
                        tile_position=(32 * m, 0),
                    )
                e = epool.tile([P, nr, ST], bf16, name=f"e_{st_i}_{g}", tag="e")
                dve = g in (DVE_G0 if st_i == 0 else DVE_G)
                if dve:
                    nc.vector.tensor_scalar(
                        e.bitcast(i16)[:], sps[:], SCHRA_A, SCHRA_B,
                        mybir.AluOpType.mult, mybir.AluOpType.add,
                    )
                else:
                    nc.scalar.activation(
                        e[:], sps[:], mybir.ActivationFunctionType.Exp
                    )
                es_by_st[st_i].append(e)

            def epilogue(st_i, ib, out_ps):
                row = st_i * 4 + ib
                inv = ivpool.tile([P, 1], fp32)
                nc.vector.reciprocal(inv, out_ps[:, C:C + 1])
                nc.vector.tensor_scalar_mul(inv, inv, gm_sb)
                stg = stpool.tile([P, C], fp32)
                nc.vector.scalar_tensor_tensor(
                    stg, out_ps[:, 0:C], inv, xpb_sb[:, row, :],
                    op0=mybir.AluOpType.mult,
                    op1=mybir.AluOpType.add,
                )
                nc.sync.dma_start(out=yT_d[ds(row * P, P), :], in_=stg[:])

            def pv_mm(st_i, ib, j, out_ps):
                es = es_by_st[st_i]
                if st_i == 0:
                    e_ap = es[j // 2][:, j % 2, ds(ib * P, P)]
                else:
                    g = min(j // 3, 10)
                    e_ap = es[g][:, j - 3 * g, ds(ib * P, P)]
                nc.tensor.matmul(
                    out_ps, e_ap, vT_sb[:, j, :],
                    start=(j == 0), stop=(j == JB - 1),
                )

            # ---- st0 head, chunk-driven: 1024-col x chunk t enables k-proj
            # quads 2t/2t+1, score pair-groups 4t..4t+3, v-proj pairs
            # 4t..4t+3; the PV of ib0/ib1 chases the exp pipeline to keep
            # the PE stream dense (HAM full-clock hold).
            chase_j = [0]      # next j for the ib0/ib1 chase (synchronized)
            done_g = [0]

            def chase(max_j):
                while chase_j[0] < min(max_j, JB):
                    j = chase_j[0]
                    pv_mm(0, 0, j, out_a)
                    pv_mm(0, 1, j, out_b)
                    chase_j[0] += 1

            for t in range(4):
                k_proj(2 * t)
                if t == 0:
                    q_proj(0)
                k_proj(2 * t + 1)
                while done_g[0] < 4 * (t + 1):
                    scores_group(0, done_g[0])
                    done_g[0] += 1
                    v_proj_pair(done_g[0] - 1)
                    # hold the chase one group behind the exp pipeline so
                    # the in-order PE queue never blocks on a fresh exp
                    chase(2 * done_g[0] - 2)

            # all remaining q chunks now (xb fully resident): keeps the q
            # bias-adds off the congested supertile boundaries
            q_proj(1)
            q_proj(2)
            q_proj(3)
            chase(JB)
            epilogue(0, 0, out_a)
            epilogue(0, 1, out_b)

            # release the head PSUM layout; steady state: triple score
            # tiles (2x3 banks) + rotating PV out pool (2 banks).
            _free_b()
            _free_a()
            head_ps.close()
            with (
                tc.tile_pool(name="pp_mm", bufs=2, space="PSUM") as pp_mm,
                tc.tile_pool(name="pp_out", bufs=2, space="PSUM") as pp_out,
            ):
                mm_pool[0] = pp_mm
                qp_pool[0] = (pp_out, "out")

                nxt = [0]
                cnt = [0]

                def weave(st_n, every):
                    cnt[0] += 1
                    if cnt[0] % every == 0 and st_n < NST and nxt[0] < NGR:
                        scores_group(st_n, nxt[0])
                        nxt[0] += 1

                # st0 ib2/ib3 run dense from the stored e-tiles, weaving
                # st1's triples (11 groups over 64 matmuls).
                for ib in (2, 3):
                    out_ps = pp_out.tile(
                        [P, C + 1], fp32, tag="out", name=f"out_0_{ib}")
                    for j in range(JB):
                        pv_mm(0, ib, j, out_ps)
                        weave(1, 5)
                    epilogue(0, ib, out_ps)
                while nxt[0] < NGR:
                    scores_group(1, nxt[0])
                    nxt[0] += 1

                # ---- st1..3 PV; next supertile's scores/exp woven in ----
                for st_i in range(1, NST):
                    nxt[0] = 0
                    cnt[0] = 0
                    for ib in range(4):
                        out_ps = pp_out.tile(
                            [P, C + 1], fp32, tag="out", name=f"out_{st_i}_{ib}")
                        for j in range(JB):
                            pv_mm(st_i, ib, j, out_ps)
                            weave(st_i + 1, 10)
                        epilogue(st_i, ib, out_ps)
                    while st_i + 1 < NST and nxt[0] < NGR:
                        scores_group(st_i + 1, nxt[0])
                        nxt[0] += 1

    return nc


def _get_program():
    global _PROG
    if _PROG is None:
        _PROG = _build_program()
        if not _PROG.is_finalized():
            _PROG.finalize()
    return _PROG


def kernel(x, Wq, bq, Wk, bk, Wv, bv, gamma):
    global LAST_RESULT
    import ml_dtypes
    from concourse.bass_utils import run_bass_kernel_spmd

    bf16 = ml_dtypes.bfloat16
    x = np.ascontiguousarray(np.asarray(x, dtype=np.float32))
    Wq = np.asarray(Wq, dtype=np.float32)
    bq = np.asarray(bq, dtype=np.float32)
    Wk = np.asarray(Wk, dtype=np.float32)
    bk = np.asarray(bk, dtype=np.float32)
    Wv = np.asarray(Wv, dtype=np.float32)
    bv = np.asarray(bv, dtype=np.float32)
    gamma = np.asarray(gamma, dtype=np.float32)

    # wq replicated into all four 32-row groups of the PE array
    wq_rep = np.zeros((C, P), dtype=np.float32)
    for r in range(4):
        wq_rep[:, 32 * r:32 * (r + 1)] = Wq.T
    gval = float(gamma.reshape(-1)[0])
    consts = np.zeros((P, 4), dtype=np.float32)
    consts[:, 0] = np.tile(bq, 4)
    consts[:, 1] = np.tile(bk, 4)
    consts[:, 2] = gval

    def _swz(a):
        # [C, F] -> [128, NCB*F]: exact SBUF layout (partition-major)
        f = a.reshape(NCB, P, -1)
        return np.ascontiguousarray(
            f.transpose(1, 0, 2).reshape(P, -1).astype(bf16)
        )

    wq_pre = _swz(wq_rep)
    wkT_pre = _swz(Wk.T)
    wv_pre = _swz(Wv.T)

    xf = x.reshape(B, C, N)
    in_maps = []
    for core in range(8):
        b, h = core // 2, core % 2
        xb = xf[b]
        if h == 0:
            x_roll = xb
        else:
            x_roll = np.concatenate([xb[:, NH:], xb[:, :NH]], axis=1)
        # xpb[p, o, c] = x_roll[c, o*128 + p] + gamma*bv[c]  (SBUF layout)
        xqT = x_roll[:, :NH].T + gval * bv[None, :]
        xpb = np.ascontiguousarray(
            xqT.reshape(NH // P, P, C).transpose(1, 0, 2).reshape(P, (NH // P) * C)
        ).astype(np.float32)
        in_maps.append({
            "xb": np.ascontiguousarray(x_roll.astype(bf16)),
            "xpb": xpb,
            "wq_pre": wq_pre,
            "wkT_pre": wkT_pre,
            "wv_pre": wv_pre,
            "consts": consts,
        })

    nc = _get_program()
    res = run_bass_kernel_spmd(
        nc, in_maps, core_ids=list(range(8)),
        trace=bool(os.environ.get("BASS_TRACE")),
    )
    LAST_RESULT = res

    out = np.empty((B, C, N), dtype=np.float32)
    for core in range(8):
        b, h = core // 2, core % 2
        yT = res.results[core]["yT"]
        out[b][:, h * NH:(h + 1) * NH] = yT.T
    return out.reshape(B, C, H, W)
